# revision 25
# baseline (speedup 1.0000x reference)
"""Trainium2 Bass kernel for nn_LocalSwarmAggregator (sliding-window causal MHA).

Reference computation (fp32):
    q,k,v = x@Wq+bq, x@Wk+bk, x@Wv+bv          # [B,N,D] -> per-head [B,H,N,64]
    logits = q k^T / 8 + band_mask              # causal + 256-window
    out = softmax(logits) v                     # [B,H,N,64]
    y = concat_heads(out) @ Wo + bo             # [B,N,D]

Sharding over 8 cores: core c handles batch c//4 and heads 4*(c%4)..4*(c%4)+3
(tensor-parallel on the head dim of Wq/Wk/Wv and the row dim of Wo).  Each
core computes a partial y for its batch; the host sums the 4 partials per
batch and adds bo.  No cross-device communication.

Per-core kernel layout (all fp32 storage):
  - x^T [D,N] built on-chip via PE transposes (projections contract over D).
  - q^T,k^T [256,N] head-pair-stacked; v^T transposed again to v natural,
    augmented with a ones column (v_aug) so the attention-weight row sums
    come out of the AV matmul for free.
  - S^T tiles [128 keys, up to 384 queries]: for key tile kt the only
    queries attending are 128*kt .. 128*kt+383, and the valid band within
    the tile is r <= c <= r+256 for every kt -> one constant 0/1 mask.
  - P^T = exp(S^T/8) * mask01 (no row-max subtraction needed: logits are
    O(6) so exp is safe in fp32).
  - AV: out^T_aug[65, q] accumulated over kt in PSUM (has_written gives
    overwrite-then-accumulate per element), row 64 = softmax denominators.
  - normalize via reciprocal + gpsimd partition_broadcast, then the output
    projection contracts head pairs (K=128) against Wo row-pairs.
"""

import os
from contextlib import ExitStack

import numpy as np

import concourse.bass as bass
import concourse.mybir as mybir
import concourse.tile as tile
from concourse import bacc
from concourse.bass_utils import run_bass_kernel_spmd
from concourse.masks import make_identity

F32 = mybir.dt.float32
N = 2048
D = 1024
HD = 64
WIN = 256
NPAIR = 2  # head pairs per core (4 heads)
NSEQT = N // 128  # 16
NDCH = D // 128  # 8
NKT = N // 128  # 16 key tiles
SPAN = 384  # max query span per S^T key tile
QG = 512  # AV / projection query group size
NQG = N // QG  # 4
SCALE = 1.0 / np.sqrt(HD)

# matmul compute dtype: float32r runs 4x faster on the PE than float32
# (single-pass reduced-precision mode vs 2-pass exact fp32).
_MM_DT_NAME = os.environ.get("BASS_MM_DT", "float32r")
MM_DT = getattr(mybir.dt, _MM_DT_NAME)

Exp = mybir.ActivationFunctionType.Exp
IS_GE = mybir.AluOpType.is_ge


def _av_slices(kt):
    """For key tile kt return [(g, lo, hi, plo)]: query-group g consumes
    P^T[kt][:, lo:hi] into psum columns plo:plo+(hi-lo)."""
    span = min(SPAN, N - 128 * kt)
    out = []
    for g in range(NQG):
        lo = max(0, QG * g - 128 * kt)
        hi = min(span, QG * g + QG - 128 * kt)
        if lo < hi:
            out.append((g, lo, hi, 128 * kt + lo - QG * g))
    return out


def _group_kts(g):
    """Key tiles contributing to query group g (ordered)."""
    return [kt for kt in range(NKT) if any(s[0] == g for s in _av_slices(kt))]


def _emit(ctx: ExitStack, tc: tile.TileContext, aps, mm_dt):
    nc = tc.nc
    x, wq, wk, wv, wo, bq, bk, bv, out = aps
    MDT = mm_dt

    def fr(ap):
        return ap

    consts = ctx.enter_context(tc.tile_pool(name="consts", bufs=1))
    persist = ctx.enter_context(tc.tile_pool(name="persist", bufs=1))

    ident_f = consts.tile([128, 128], F32, tag="ident_f")
    make_identity(nc, ident_f)
    ident = consts.tile([128, 128], MDT, tag="ident")
    nc.vector.tensor_copy(ident, ident_f)

    # 0/1 band mask: valid iff r <= c <= r + WIN  (keys on partitions,
    # query offset on free dim)
    mask = consts.tile([128, SPAN], F32, tag="mask")
    mask2 = consts.tile([128, 2, SPAN], F32, tag="mask2")
    nc.gpsimd.memset(mask, 1.0)
    nc.gpsimd.affine_select(
        out=mask, in_=mask, compare_op=IS_GE, fill=0.0,
        base=0, pattern=[[1, SPAN]], channel_multiplier=-1,
    )  # keep c - r >= 0
    nc.gpsimd.affine_select(
        out=mask, in_=mask, compare_op=IS_GE, fill=0.0,
        base=WIN, pattern=[[-1, SPAN]], channel_multiplier=1,
    )  # keep r - c + WIN >= 0
    nc.gpsimd.tensor_copy(mask2[:, 0, :], mask)
    nc.gpsimd.tensor_copy(mask2[:, 1, :], mask)

    # zero / ones helpers (DVE-written so fp32r versions count as rounded)
    zf = consts.tile([1, QG], F32, tag="zf")
    nc.vector.memset(zf, 0.0)
    onesf = consts.tile([128, 1], F32, tag="onesf")
    nc.vector.memset(onesf, 1.0)
    zcol = consts.tile([1, 65], MDT, tag="zcol")
    nc.vector.tensor_copy(zcol, zf[:, 0:65])
    zrow = consts.tile([1, QG], MDT, tag="zrow")
    nc.vector.tensor_copy(zrow, zf)

    # persistent intermediates
    qT = persist.tile([128, NPAIR, N], MDT, tag="qT")  # 16KB
    kT = persist.tile([128, NPAIR, N], MDT, tag="kT")  # 16KB
    vaug = [persist.tile([128, NKT, 2, HD + 1], MDT, tag=f"vaug{p}",
                         name=f"vaug{p}")
            for p in range(NPAIR)]  # 8.3KB each
    U2 = persist.tile([128, NPAIR, N], MDT, tag="U2")  # 16KB

    # ---------------- phase A: x^T + QKV projections + v_aug ----------------
    with ExitStack() as pha:
        xn_pool = pha.enter_context(tc.tile_pool(name="xn", bufs=4))
        xt_pool = pha.enter_context(tc.tile_pool(name="xt", bufs=3))
        vt_pool = pha.enter_context(tc.tile_pool(name="vt", bufs=1))
        psA = pha.enter_context(tc.tile_pool(name="psA", bufs=4, space="PSUM"))
        psQ = pha.enter_context(tc.tile_pool(name="psQ", bufs=4, space="PSUM"))

        vT = vt_pool.tile([128, NPAIR, N], MDT, tag="vT")
        w_sb = {}
        b_sb = {}

        for g in range(NQG):
            # x^T for this query group, built via PE transposes
            xTg = xt_pool.tile([128, NDCH, QG], MDT, tag="xTg")
            for si in range(4):
                s = 4 * g + si
                xn = xn_pool.tile([128, D], MDT, tag="xn")
                if s < 2:
                    nc.sync.dma_start(out=xn[:, 0:QG],
                                      in_=x[128 * s:128 * (s + 1), 0:QG])
                    nc.sync.dma_start(out=xn[:, QG:D],
                                      in_=x[128 * s:128 * (s + 1), QG:D])
                else:
                    nc.sync.dma_start(out=xn, in_=x[128 * s:128 * (s + 1), :])
                for dh in range(2):
                    ps = psA.tile([128, QG], MDT, tag="psA")
                    for dj in range(4):
                        d = 4 * dh + dj
                        nc.tensor.transpose(
                            ps[:, 128 * dj:128 * (dj + 1)],
                            xn[:, 128 * d:128 * (d + 1)], ident,
                        )
                    cp = nc.scalar.copy if (si + dh) % 2 == 0 \
                        else nc.vector.tensor_copy
                    cp(xTg[:, 4 * dh:4 * (dh + 1), 128 * si:128 * (si + 1)],
                       ps.rearrange("p (c q) -> p c q", q=128))

            if g == 0:
                # weights + biases (emitted after the first x tiles so x
                # wins the DMA queues; only QKV matmuls gate on these)
                for nm, wap in (("q", wq), ("k", wk), ("v", wv)):
                    t = consts.tile([128, NDCH, 2 * 128], MDT, tag=f"w{nm}",
                                    name=f"w{nm}")
                    nc.sync.dma_start(
                        out=t, in_=wap.rearrange("(c p) m -> p c m", p=128))
                    w_sb[nm] = t
                wo_sb = consts.tile([128, NPAIR, D], MDT, tag="wo")
                nc.sync.dma_start(
                    out=wo_sb, in_=wo.rearrange("(pair p) m -> p pair m", p=128))
                for nm, bap in (("q", bq), ("k", bk), ("v", bv)):
                    t = consts.tile([128, NPAIR], F32, tag=f"b{nm}",
                                    name=f"b{nm}")
                    nc.sync.dma_start(
                        out=t, in_=bap.rearrange("(pair p) -> p pair", p=128))
                    b_sb[nm] = t

            for pair in range(NPAIR):
                for nm, dstT in (("q", qT), ("k", kT), ("v", vT)):
                    psq = psQ.tile([128, QG], F32, tag="psQ")
                    for d in range(NDCH):
                        nc.tensor.matmul(
                            psq,
                            fr(w_sb[nm][:, d, 128 * pair:128 * (pair + 1)]),
                            fr(xTg[:, d, :]),
                            start=(d == 0), stop=(d == NDCH - 1),
                        )
                    nc.vector.tensor_scalar_add(
                        dstT[:, pair, QG * g:QG * (g + 1)], psq,
                        b_sb[nm][:, pair:pair + 1],
                    )

        # v natural (+ ones col)
        for pair in range(NPAIR):
            nc.vector.tensor_copy(
                vaug[pair][:, :, :, HD:HD + 1],
                onesf.broadcast_to((128, NKT, 2, 1)),
            )
            for quarter in range(4):
                ps = psA.tile([128, QG], MDT, tag="psA")
                for j in range(4):
                    kt = 4 * quarter + j
                    nc.tensor.transpose(
                        ps[:, 128 * j:128 * (j + 1)],
                        vT[:, pair, 128 * kt:128 * (kt + 1)], ident,
                    )
                cp = nc.scalar.copy if quarter % 2 == 0 \
                    else nc.vector.tensor_copy
                cp(
                    vaug[pair][:, 4 * quarter:4 * (quarter + 1), :, 0:HD],
                    ps.rearrange("p (j h d) -> p j h d", j=4, h=2),
                )

    # ------- phase B+C: attention (kg-pipelined) with interleaved out-proj -----
    with ExitStack() as phb:
        psS = phb.enter_context(tc.tile_pool(name="psS", bufs=2, space="PSUM"))
        psAVO = phb.enter_context(tc.tile_pool(name="psAVO", bufs=4, space="PSUM"))
        pt_pool = phb.enter_context(tc.tile_pool(name="pt", bufs=12))
        rb_pool = phb.enter_context(tc.tile_pool(name="rb", bufs=3))
        ob_pool = phb.enter_context(tc.tile_pool(name="ob", bufs=5))

        first_kt = {g: _group_kts(g)[0] for g in range(NQG)}
        last_kt = {g: _group_kts(g)[-1] for g in range(NQG)}

        def outproj_group(g):
            for qt in range(4 * g, 4 * (g + 1)):
                for dh in range(2):
                    pso = psAVO.tile([128, QG], F32, tag="avO", name="pso")
                    for pair in range(NPAIR):
                        nc.tensor.matmul(
                            pso,
                            U2[:, pair, 128 * qt:128 * (qt + 1)],
                            wo_sb[:, pair, QG * dh:QG * (dh + 1)],
                            start=(pair == 0), stop=(pair == NPAIR - 1),
                        )
                    ob = ob_pool.tile([128, QG], F32, tag="ob")
                    cp = nc.vector.tensor_copy if (qt + dh) % 2 == 0 \
                        else nc.scalar.copy
                    cp(ob, pso)
                    nc.sync.dma_start(
                        out=out[128 * qt:128 * (qt + 1), QG * dh:QG * (dh + 1)],
                        in_=ob,
                    )

        def emit_av_group(pair, g, pts):
            """AV + normalization for query group g, both heads; pts maps
            (h, kt) -> (pt_tile, j)."""
            for h in range(2):
                psav = psAVO.tile([65, QG], F32, tag="avO", name="psav")
                nc.tensor.matmul(psav, zcol, zrow, start=True, stop=False)
                for kt in _group_kts(g):
                    pt, j = pts[(h, kt)]
                    lo, hi, plo = next(
                        (s[1], s[2], s[3]) for s in _av_slices(kt) if s[0] == g)
                    nc.tensor.matmul(
                        psav[:, plo:plo + (hi - lo)],
                        vaug[pair][:, kt, h, :],
                        pt[:, j, lo:hi],
                        start=False, stop=(kt == last_kt[g]),
                    )
                rt0 = rb_pool.tile([1, QG], F32, tag="rt0")
                nc.scalar.copy(rt0, psav[64:65, :])
                rtmp = rb_pool.tile([1, QG], F32, tag="rtmp")
                nc.vector.reciprocal_approx_fast(out=rtmp, in_=rt0)
                rbt = rb_pool.tile([64, QG], F32, tag="rb")
                nc.gpsimd.partition_broadcast(rbt, rtmp)
                nc.vector.tensor_mul(
                    U2[64 * h:64 * (h + 1), pair, QG * g:QG * (g + 1)],
                    psav[0:64, :], rbt,
                )
            if pair == NPAIR - 1:
                outproj_group(g)

        for pair in range(NPAIR):
            pts = {}
            for kg in range(NKT // 2):
                kts = [2 * kg, 2 * kg + 1]
                pss = [psS.tile([128, 2, QG], F32, tag="psS", name="pss")
                       for _ in range(2)]
                for j, kt in enumerate(kts):
                    q0 = 128 * kt
                    span = min(SPAN, N - q0)
                    for h in range(2):
                        hb = 64 * h
                        nc.tensor.matmul(
                            pss[h][:, j, 0:span],
                            kT[hb:hb + 64, pair, q0:q0 + 128],
                            qT[hb:hb + 64, pair, q0:q0 + span],
                            start=True, stop=True,
                        )
                for h in range(2):
                    pt = pt_pool.tile([128, 2, SPAN], MDT, tag="pt")
                    if kg < NKT // 2 - 1:
                        nc.scalar.activation(
                            pt[:, :, :], pss[h][:, :, 0:SPAN], Exp, scale=SCALE
                        )
                        nc.vector.tensor_mul(
                            pt[:, :, 0:128], pt[:, :, 0:128],
                            mask2[:, :, 0:128],
                        )
                        nc.vector.tensor_mul(
                            pt[:, :, WIN:SPAN], pt[:, :, WIN:SPAN],
                            mask2[:, :, WIN:SPAN],
                        )
                    else:
                        for j, kt in enumerate(kts):
                            span = min(SPAN, N - 128 * kt)
                            nc.scalar.activation(
                                pt[:, j, 0:span], pss[h][:, j, 0:span], Exp,
                                scale=SCALE,
                            )
                            nc.vector.tensor_mul(
                                pt[:, j, 0:128], pt[:, j, 0:128], mask[:, 0:128]
                            )
                            if span > WIN:
                                nc.vector.tensor_mul(
                                    pt[:, j, WIN:span], pt[:, j, WIN:span],
                                    mask[:, WIN:span],
                                )
                    for j, kt in enumerate(kts):
                        pts[(h, kt)] = (pt, j)
                if kg % 2 == 1:
                    emit_av_group(pair, (kg - 1) // 2, pts)


def build(mm_dt=MM_DT):
    nc = bacc.Bacc("TRN2", target_bir_lowering=False, debug=False)
    x = nc.dram_tensor("x", [N, D], mm_dt, kind="ExternalInput").ap()
    wq = nc.dram_tensor("wq", [D, 256], mm_dt, kind="ExternalInput").ap()
    wk = nc.dram_tensor("wk", [D, 256], mm_dt, kind="ExternalInput").ap()
    wv = nc.dram_tensor("wv", [D, 256], mm_dt, kind="ExternalInput").ap()
    wo = nc.dram_tensor("wo", [256, D], mm_dt, kind="ExternalInput").ap()
    bq = nc.dram_tensor("bq", [256], F32, kind="ExternalInput").ap()
    bk = nc.dram_tensor("bk", [256], F32, kind="ExternalInput").ap()
    bv = nc.dram_tensor("bv", [256], F32, kind="ExternalInput").ap()
    out = nc.dram_tensor("out", [N, D], F32, kind="ExternalOutput").ap()
    with tile.TileContext(nc) as tc, ExitStack() as ctx:
        _emit(ctx, tc, (x, wq, wk, wv, wo, bq, bk, bv, out), mm_dt)
    nc.compile()
    return nc


def shard_inputs(x, Wq, bq, Wk, bk, Wv, bv, Wo, bo):
    """Full inputs -> list of 8 per-core input maps."""
    in_maps = []
    for c in range(8):
        b, hg = c // 4, c % 4
        cs = slice(256 * hg, 256 * (hg + 1))
        in_maps.append({
            "x": np.ascontiguousarray(x[b]),
            "wq": np.ascontiguousarray(Wq[:, cs]),
            "wk": np.ascontiguousarray(Wk[:, cs]),
            "wv": np.ascontiguousarray(Wv[:, cs]),
            "wo": np.ascontiguousarray(Wo[cs, :]),
            "bq": np.ascontiguousarray(bq[cs]),
            "bk": np.ascontiguousarray(bk[cs]),
            "bv": np.ascontiguousarray(bv[cs]),
        })
    return in_maps


def assemble(results, bo):
    """8 per-core partial outputs -> full [2, N, D] output."""
    outs = [np.asarray(r["out"], dtype=np.float32) for r in results]
    full = np.empty((2, N, D), dtype=np.float32)
    for b in range(2):
        full[b] = outs[4 * b] + outs[4 * b + 1] + outs[4 * b + 2] + outs[4 * b + 3]
        full[b] += bo[None, :]
    return full


_NC_CACHE = {}


def _get_nc():
    key = _MM_DT_NAME
    if key not in _NC_CACHE:
        _NC_CACHE[key] = build()
    return _NC_CACHE[key]


def kernel(x, Wq, bq, Wk, bk, Wv, bv, Wo, bo, _trace=False):
    x, Wq, bq, Wk, bk, Wv, bv, Wo, bo = (
        np.asarray(a, dtype=np.float32)
        for a in (x, Wq, bq, Wk, bk, Wv, bv, Wo, bo)
    )
    nc = _get_nc()
    in_maps = shard_inputs(x, Wq, bq, Wk, bk, Wv, bv, Wo, bo)
    res = run_bass_kernel_spmd(nc, in_maps, core_ids=list(range(8)), trace=_trace)
    full = assemble(res.results, bo)
    if _trace:
        kernel.last_result = res
    return full


# revision 26
# speedup vs baseline: 1.0695x; 1.0695x over previous
"""Trainium2 Bass kernel for nn_LocalSwarmAggregator (sliding-window causal MHA).

Reference computation (fp32):
    q,k,v = x@Wq+bq, x@Wk+bk, x@Wv+bv          # [B,N,D] -> per-head [B,H,N,64]
    logits = q k^T / 8 + band_mask              # causal + 256-window
    out = softmax(logits) v                     # [B,H,N,64]
    y = concat_heads(out) @ Wo + bo             # [B,N,D]

Sharding over 8 cores: core c handles batch c//4 and heads 4*(c%4)..4*(c%4)+3
(tensor-parallel on the head dim of Wq/Wk/Wv and the row dim of Wo).  Each
core computes a partial y for its batch; the host sums the 4 partials per
batch and adds bo.  No cross-device communication.

Per-core kernel layout (all fp32 storage):
  - x^T [D,N] built on-chip via PE transposes (projections contract over D).
  - q^T,k^T [256,N] head-pair-stacked; v^T transposed again to v natural,
    augmented with a ones column (v_aug) so the attention-weight row sums
    come out of the AV matmul for free.
  - S^T tiles [128 keys, up to 384 queries]: for key tile kt the only
    queries attending are 128*kt .. 128*kt+383, and the valid band within
    the tile is r <= c <= r+256 for every kt -> one constant 0/1 mask.
  - P^T = exp(S^T/8) * mask01 (no row-max subtraction needed: logits are
    O(6) so exp is safe in fp32).
  - AV: out^T_aug[65, q] accumulated over kt in PSUM (has_written gives
    overwrite-then-accumulate per element), row 64 = softmax denominators.
  - normalize via reciprocal + gpsimd partition_broadcast, then the output
    projection contracts head pairs (K=128) against Wo row-pairs.
"""

import os
from contextlib import ExitStack

import numpy as np

import concourse.bass as bass
import concourse.mybir as mybir
import concourse.tile as tile
from concourse import bacc
from concourse.bass_utils import run_bass_kernel_spmd
from concourse.masks import make_identity

F32 = mybir.dt.float32
N = 2048
D = 1024
HD = 64
WIN = 256
NPAIR = 2  # head pairs per core (4 heads)
NSEQT = N // 128  # 16
NDCH = D // 128  # 8
NKT = N // 128  # 16 key tiles
SPAN = 384  # max query span per S^T key tile
QG = 512  # AV / projection query group size
NQG = N // QG  # 4
SCALE = 1.0 / np.sqrt(HD)

# matmul compute dtype: float32r runs 4x faster on the PE than float32
# (single-pass reduced-precision mode vs 2-pass exact fp32).
_MM_DT_NAME = os.environ.get("BASS_MM_DT", "float32r")
MM_DT = getattr(mybir.dt, _MM_DT_NAME)

Exp = mybir.ActivationFunctionType.Exp
IS_GE = mybir.AluOpType.is_ge


def _av_slices(kt):
    """For key tile kt return [(g, lo, hi, plo)]: query-group g consumes
    P^T[kt][:, lo:hi] into psum columns plo:plo+(hi-lo)."""
    span = min(SPAN, N - 128 * kt)
    out = []
    for g in range(NQG):
        lo = max(0, QG * g - 128 * kt)
        hi = min(span, QG * g + QG - 128 * kt)
        if lo < hi:
            out.append((g, lo, hi, 128 * kt + lo - QG * g))
    return out


def _group_kts(g):
    """Key tiles contributing to query group g (ordered)."""
    return [kt for kt in range(NKT) if any(s[0] == g for s in _av_slices(kt))]


def _emit(ctx: ExitStack, tc: tile.TileContext, aps, mm_dt):
    nc = tc.nc
    x, wq, wk, wv, wo, bq, bk, bv, out = aps
    MDT = mm_dt

    def fr(ap):
        return ap

    consts = ctx.enter_context(tc.tile_pool(name="consts", bufs=1))
    persist = ctx.enter_context(tc.tile_pool(name="persist", bufs=1))

    ident_f = consts.tile([128, 128], F32, tag="ident_f")
    make_identity(nc, ident_f)
    ident = consts.tile([128, 128], MDT, tag="ident")
    nc.vector.tensor_copy(ident, ident_f)

    # 0/1 band mask: valid iff r <= c <= r + WIN  (keys on partitions,
    # query offset on free dim)
    mask = consts.tile([128, SPAN], F32, tag="mask")
    mask2 = consts.tile([128, 2, SPAN], F32, tag="mask2")
    nc.gpsimd.memset(mask, 1.0)
    nc.gpsimd.affine_select(
        out=mask, in_=mask, compare_op=IS_GE, fill=0.0,
        base=0, pattern=[[1, SPAN]], channel_multiplier=-1,
    )  # keep c - r >= 0
    nc.gpsimd.affine_select(
        out=mask, in_=mask, compare_op=IS_GE, fill=0.0,
        base=WIN, pattern=[[-1, SPAN]], channel_multiplier=1,
    )  # keep r - c + WIN >= 0
    nc.gpsimd.tensor_copy(mask2[:, 0, :], mask)
    nc.gpsimd.tensor_copy(mask2[:, 1, :], mask)

    # zero / ones helpers (DVE-written so fp32r versions count as rounded)
    zf = consts.tile([1, QG], F32, tag="zf")
    nc.vector.memset(zf, 0.0)
    onesf = consts.tile([128, 1], F32, tag="onesf")
    nc.vector.memset(onesf, 1.0)
    zcol = consts.tile([1, 65], MDT, tag="zcol")
    nc.vector.tensor_copy(zcol, zf[:, 0:65])
    zrow = consts.tile([1, QG], MDT, tag="zrow")
    nc.vector.tensor_copy(zrow, zf)

    # persistent intermediates
    qT = persist.tile([128, NPAIR, N], MDT, tag="qT")  # 16KB
    kT = persist.tile([128, NPAIR, N], MDT, tag="kT")  # 16KB
    vaug = [persist.tile([128, NKT, 2, HD + 1], MDT, tag=f"vaug{p}",
                         name=f"vaug{p}")
            for p in range(NPAIR)]  # 8.3KB each
    U2 = persist.tile([128, NPAIR, N], MDT, tag="U2")  # 16KB

    # ---------------- phase A: x^T + QKV projections + v_aug ----------------
    with ExitStack() as pha:
        xn_pool = pha.enter_context(tc.tile_pool(name="xn", bufs=4))
        xt_pool = pha.enter_context(tc.tile_pool(name="xt", bufs=3))
        vt_pool = pha.enter_context(tc.tile_pool(name="vt", bufs=1))
        psA = pha.enter_context(tc.tile_pool(name="psA", bufs=4, space="PSUM"))
        psQ = pha.enter_context(tc.tile_pool(name="psQ", bufs=4, space="PSUM"))

        vT = vt_pool.tile([128, NPAIR, N], MDT, tag="vT")
        w_sb = {}
        b_sb = {}

        for g in range(NQG):
            # x^T for this query group, built via PE transposes
            xTg = xt_pool.tile([128, NDCH, QG], MDT, tag="xTg")
            for si in range(4):
                s = 4 * g + si
                xn = xn_pool.tile([128, D], MDT, tag="xn")
                nc.sync.dma_start(out=xn, in_=x[128 * s:128 * (s + 1), :])
                for dh in range(2):
                    ps = psA.tile([128, QG], MDT, tag="psA")
                    for dj in range(4):
                        d = 4 * dh + dj
                        nc.tensor.transpose(
                            ps[:, 128 * dj:128 * (dj + 1)],
                            xn[:, 128 * d:128 * (d + 1)], ident,
                        )
                    cp = nc.scalar.copy if (si + dh) % 2 == 0 \
                        else nc.vector.tensor_copy
                    cp(xTg[:, 4 * dh:4 * (dh + 1), 128 * si:128 * (si + 1)],
                       ps.rearrange("p (c q) -> p c q", q=128))

            if g == 0:
                # weights + biases (emitted after the first x tiles so x
                # wins the DMA queues; only QKV matmuls gate on these)
                for nm, wap in (("q", wq), ("k", wk), ("v", wv)):
                    t = consts.tile([128, NDCH, 2 * 128], MDT, tag=f"w{nm}",
                                    name=f"w{nm}")
                    nc.sync.dma_start(
                        out=t, in_=wap.rearrange("(c p) m -> p c m", p=128))
                    w_sb[nm] = t
                wo_sb = consts.tile([128, NPAIR, D], MDT, tag="wo")
                nc.sync.dma_start(
                    out=wo_sb, in_=wo.rearrange("(pair p) m -> p pair m", p=128))
                for nm, bap in (("q", bq), ("k", bk), ("v", bv)):
                    t = consts.tile([128, NPAIR], F32, tag=f"b{nm}",
                                    name=f"b{nm}")
                    nc.sync.dma_start(
                        out=t, in_=bap.rearrange("(pair p) -> p pair", p=128))
                    b_sb[nm] = t

            for pair in range(NPAIR):
                for nm, dstT in (("q", qT), ("k", kT), ("v", vT)):
                    psq = psQ.tile([128, QG], F32, tag="psQ")
                    for d in range(NDCH):
                        nc.tensor.matmul(
                            psq,
                            fr(w_sb[nm][:, d, 128 * pair:128 * (pair + 1)]),
                            fr(xTg[:, d, :]),
                            start=(d == 0), stop=(d == NDCH - 1),
                        )
                    nc.vector.tensor_scalar_add(
                        dstT[:, pair, QG * g:QG * (g + 1)], psq,
                        b_sb[nm][:, pair:pair + 1],
                    )

        # v natural (+ ones col)
        for pair in range(NPAIR):
            nc.vector.tensor_copy(
                vaug[pair][:, :, :, HD:HD + 1],
                onesf.broadcast_to((128, NKT, 2, 1)),
            )
            for quarter in range(4):
                ps = psA.tile([128, QG], MDT, tag="psA")
                for j in range(4):
                    kt = 4 * quarter + j
                    nc.tensor.transpose(
                        ps[:, 128 * j:128 * (j + 1)],
                        vT[:, pair, 128 * kt:128 * (kt + 1)], ident,
                    )
                cp = nc.scalar.copy if quarter % 2 == 0 \
                    else nc.vector.tensor_copy
                cp(
                    vaug[pair][:, 4 * quarter:4 * (quarter + 1), :, 0:HD],
                    ps.rearrange("p (j h d) -> p j h d", j=4, h=2),
                )

    # ------- phase B+C: attention (kg-pipelined) with interleaved out-proj -----
    with ExitStack() as phb:
        psS = phb.enter_context(tc.tile_pool(name="psS", bufs=2, space="PSUM"))
        psAV = phb.enter_context(tc.tile_pool(name="psAV", bufs=2, space="PSUM"))
        psO = phb.enter_context(tc.tile_pool(name="psO", bufs=2, space="PSUM"))
        pt_pool = phb.enter_context(tc.tile_pool(name="pt", bufs=12))
        rb_pool = phb.enter_context(tc.tile_pool(name="rb", bufs=3))
        ob_pool = phb.enter_context(tc.tile_pool(name="ob", bufs=5))

        first_kt = {g: _group_kts(g)[0] for g in range(NQG)}
        last_kt = {g: _group_kts(g)[-1] for g in range(NQG)}

        def outproj_group(g):
            for qt in range(4 * g, 4 * (g + 1)):
                for dh in range(2):
                    pso = psO.tile([128, QG], F32, tag="psO", name="pso")
                    for pair in range(NPAIR):
                        nc.tensor.matmul(
                            pso,
                            U2[:, pair, 128 * qt:128 * (qt + 1)],
                            wo_sb[:, pair, QG * dh:QG * (dh + 1)],
                            start=(pair == 0), stop=(pair == NPAIR - 1),
                        )
                    ob = ob_pool.tile([128, QG], F32, tag="ob")
                    cp = nc.vector.tensor_copy if (qt + dh) % 2 == 0 \
                        else nc.scalar.copy
                    cp(ob, pso)
                    nc.sync.dma_start(
                        out=out[128 * qt:128 * (qt + 1), QG * dh:QG * (dh + 1)],
                        in_=ob,
                    )

        def emit_av_group(pair, g, pts):
            """AV + normalization for query group g, both heads; pts maps
            (h, kt) -> (pt_tile, j)."""
            for h in range(2):
                psav = psAV.tile([65, QG], F32, tag="psAV", name="psav")
                nc.tensor.matmul(psav, zcol, zrow, start=True, stop=False)
                for kt in _group_kts(g):
                    pt, j = pts[(h, kt)]
                    lo, hi, plo = next(
                        (s[1], s[2], s[3]) for s in _av_slices(kt) if s[0] == g)
                    nc.tensor.matmul(
                        psav[:, plo:plo + (hi - lo)],
                        vaug[pair][:, kt, h, :],
                        pt[:, j, lo:hi],
                        start=False, stop=(kt == last_kt[g]),
                    )
                rt0 = rb_pool.tile([1, QG], F32, tag="rt0")
                nc.scalar.copy(rt0, psav[64:65, :])
                rtmp = rb_pool.tile([1, QG], F32, tag="rtmp")
                nc.vector.reciprocal_approx_fast(out=rtmp, in_=rt0)
                rbt = rb_pool.tile([64, QG], F32, tag="rb")
                nc.gpsimd.partition_broadcast(rbt, rtmp)
                nc.vector.tensor_mul(
                    U2[64 * h:64 * (h + 1), pair, QG * g:QG * (g + 1)],
                    psav[0:64, :], rbt,
                )
            if pair == NPAIR - 1:
                outproj_group(g)

        for pair in range(NPAIR):
            pts = {}
            for kg in range(NKT // 2):
                kts = [2 * kg, 2 * kg + 1]
                pss = [psS.tile([128, 2, QG], F32, tag="psS", name="pss")
                       for _ in range(2)]
                for j, kt in enumerate(kts):
                    q0 = 128 * kt
                    span = min(SPAN, N - q0)
                    for h in range(2):
                        hb = 64 * h
                        nc.tensor.matmul(
                            pss[h][:, j, 0:span],
                            kT[hb:hb + 64, pair, q0:q0 + 128],
                            qT[hb:hb + 64, pair, q0:q0 + span],
                            start=True, stop=True,
                        )
                for h in range(2):
                    pt = pt_pool.tile([128, 2, SPAN], MDT, tag="pt")
                    if kg < NKT // 2 - 1:
                        nc.scalar.activation(
                            pt[:, :, :], pss[h][:, :, 0:SPAN], Exp, scale=SCALE
                        )
                        nc.vector.tensor_mul(
                            pt[:, :, 0:128], pt[:, :, 0:128],
                            mask2[:, :, 0:128],
                        )
                        nc.vector.tensor_mul(
                            pt[:, :, WIN:SPAN], pt[:, :, WIN:SPAN],
                            mask2[:, :, WIN:SPAN],
                        )
                    else:
                        for j, kt in enumerate(kts):
                            span = min(SPAN, N - 128 * kt)
                            nc.scalar.activation(
                                pt[:, j, 0:span], pss[h][:, j, 0:span], Exp,
                                scale=SCALE,
                            )
                            nc.vector.tensor_mul(
                                pt[:, j, 0:128], pt[:, j, 0:128], mask[:, 0:128]
                            )
                            if span > WIN:
                                nc.vector.tensor_mul(
                                    pt[:, j, WIN:span], pt[:, j, WIN:span],
                                    mask[:, WIN:span],
                                )
                    for j, kt in enumerate(kts):
                        pts[(h, kt)] = (pt, j)
                if kg % 2 == 1:
                    emit_av_group(pair, (kg - 1) // 2, pts)


def build(mm_dt=MM_DT):
    nc = bacc.Bacc("TRN2", target_bir_lowering=False, debug=False)
    x = nc.dram_tensor("x", [N, D], mm_dt, kind="ExternalInput").ap()
    wq = nc.dram_tensor("wq", [D, 256], mm_dt, kind="ExternalInput").ap()
    wk = nc.dram_tensor("wk", [D, 256], mm_dt, kind="ExternalInput").ap()
    wv = nc.dram_tensor("wv", [D, 256], mm_dt, kind="ExternalInput").ap()
    wo = nc.dram_tensor("wo", [256, D], mm_dt, kind="ExternalInput").ap()
    bq = nc.dram_tensor("bq", [256], F32, kind="ExternalInput").ap()
    bk = nc.dram_tensor("bk", [256], F32, kind="ExternalInput").ap()
    bv = nc.dram_tensor("bv", [256], F32, kind="ExternalInput").ap()
    out = nc.dram_tensor("out", [N, D], F32, kind="ExternalOutput").ap()
    with tile.TileContext(nc) as tc, ExitStack() as ctx:
        _emit(ctx, tc, (x, wq, wk, wv, wo, bq, bk, bv, out), mm_dt)
    nc.compile()
    return nc


def shard_inputs(x, Wq, bq, Wk, bk, Wv, bv, Wo, bo):
    """Full inputs -> list of 8 per-core input maps."""
    in_maps = []
    for c in range(8):
        b, hg = c // 4, c % 4
        cs = slice(256 * hg, 256 * (hg + 1))
        in_maps.append({
            "x": np.ascontiguousarray(x[b]),
            "wq": np.ascontiguousarray(Wq[:, cs]),
            "wk": np.ascontiguousarray(Wk[:, cs]),
            "wv": np.ascontiguousarray(Wv[:, cs]),
            "wo": np.ascontiguousarray(Wo[cs, :]),
            "bq": np.ascontiguousarray(bq[cs]),
            "bk": np.ascontiguousarray(bk[cs]),
            "bv": np.ascontiguousarray(bv[cs]),
        })
    return in_maps


def assemble(results, bo):
    """8 per-core partial outputs -> full [2, N, D] output."""
    outs = [np.asarray(r["out"], dtype=np.float32) for r in results]
    full = np.empty((2, N, D), dtype=np.float32)
    for b in range(2):
        full[b] = outs[4 * b] + outs[4 * b + 1] + outs[4 * b + 2] + outs[4 * b + 3]
        full[b] += bo[None, :]
    return full


_NC_CACHE = {}


def _get_nc():
    key = _MM_DT_NAME
    if key not in _NC_CACHE:
        _NC_CACHE[key] = build()
    return _NC_CACHE[key]


def kernel(x, Wq, bq, Wk, bk, Wv, bv, Wo, bo, _trace=False):
    x, Wq, bq, Wk, bk, Wv, bv, Wo, bo = (
        np.asarray(a, dtype=np.float32)
        for a in (x, Wq, bq, Wk, bk, Wv, bv, Wo, bo)
    )
    nc = _get_nc()
    in_maps = shard_inputs(x, Wq, bq, Wk, bk, Wv, bv, Wo, bo)
    res = run_bass_kernel_spmd(nc, in_maps, core_ids=list(range(8)), trace=_trace)
    full = assemble(res.results, bo)
    if _trace:
        kernel.last_result = res
    return full
